# revision 2
# baseline (speedup 1.0000x reference)
"""DeepSeekMoE layer on 8 Trainium2 NeuronCores, expert-parallel.

Contract: kernel(**inputs) takes the FULL unsharded inputs (numpy, f32) and
returns the FULL output [1, 1024, 2048] f32.

Strategy
--------
- Routing gate computed on host (67 MFLOP — negligible); it determines the
  token->expert dispatch, i.e. the sharding of the device work.
- 16 routed experts paired big+small onto 8 cores (expert parallelism).
  Each core runs a 3-matmul SwiGLU MLP per expert over only the tokens
  routed to that expert (top-4 of 16 => ~256 tokens/expert).
- Shared experts are tensor-sharded: each core owns a 352-wide (padded to
  384) slice of the 2816 shared intermediate dim and computes a partial
  down-projection over all 1024 tokens; host sums partials.
- All device matmuls in bf16 (f32 PSUM accumulation). Weights are
  transposed/packed on host into exact consumption order so every DMA is a
  single contiguous block.
"""

import os
import sys
import types
import contextlib
from contextlib import ExitStack

import numpy as np
import ml_dtypes

for _p in ("/opt/trn_rl_repo",):
    if _p not in sys.path and os.path.isdir(_p):
        sys.path.append(_p)

import concourse.bass as bass
import concourse.mybir as mybir
import concourse.tile as tile
from concourse.bass_utils import run_bass_kernel_spmd

BF16 = ml_dtypes.bfloat16

# Problem constants (hardcoded per contract)
B, S, H = 1, 1024, 2048
T = B * S
I = 1408
E = 16
TOP_K = 4
N_GROUP = 4
TOPK_GROUP = 2
SCALE = 2.5
SI = 2816               # shared intermediate
N_CORES = 8
H_TILES = H // 128      # 16
I_TILES = I // 128      # 11
SH_SLICE = SI // N_CORES        # 352
SH_PAD = 384                    # padded to 3 i-tiles
SH_I_TILES = SH_PAD // 128      # 3

LAST_EXEC_TIME_NS = None


# ----------------------------------------------------------------------------
# axon NTFF profile hook shim (lets run_bass_kernel_spmd(trace=True) work)
# ----------------------------------------------------------------------------
def _install_axon_profile_shim():
    try:
        import antenv
        try:
            from antenv.axon_hooks import get_axon_ntff_profile_hook  # noqa
            return True  # already present
        except ImportError:
            pass
        _state = {"hook": None}

        def set_axon_ntff_profile_hook(h):
            _state["hook"] = h

        def get_axon_ntff_profile_hook():
            return _state["hook"]

        mod = types.ModuleType("antenv.axon_hooks")
        mod.set_axon_ntff_profile_hook = set_axon_ntff_profile_hook
        mod.get_axon_ntff_profile_hook = get_axon_ntff_profile_hook
        sys.modules["antenv.axon_hooks"] = mod
        antenv.axon_hooks = mod
        from trn_agent_boot.trn_boot import _ntff_profile_via_ctypes
        hook = _ntff_profile_via_ctypes("/opt/axon/libaxon_pjrt.so")
        set_axon_ntff_profile_hook(hook)
        return hook is not None
    except Exception:
        return False


# ----------------------------------------------------------------------------
# walrus workaround: split >1 semaphore waits per instruction onto NoOps
# ----------------------------------------------------------------------------
def _split_excess_waits(nc, max_waits=1):
    n = 0
    for f in nc.m.functions:
        for bb in f.blocks:
            new = []
            for inst in bb.instructions:
                si = getattr(inst, "sync_info", None)
                if si is not None and si.on_wait and len(si.on_wait) > max_waits:
                    waits = list(si.on_wait)
                    si.on_wait = waits[:max_waits]
                    for ci, w in enumerate(waits[max_waits:]):
                        nop = mybir.InstNoOp(
                            name=f"{inst.name}_ws{ci}",
                            sync_info=mybir.SyncInfo(on_wait=[w], on_update=[]),
                            bass_nofuse=True,
                            engine=inst.engine,
                        )
                        new.append(nop)
                        n += 1
                new.append(inst)
            bb.instructions[:] = new
    return n


# ----------------------------------------------------------------------------
# Host-side gate (replicates reference._gate routing decisions in numpy)
# ----------------------------------------------------------------------------
def _gate_host(x, gate_w, gate_bias):
    logits = x.astype(np.float32) @ gate_w.astype(np.float32).T        # [T,E]
    scores = 1.0 / (1.0 + np.exp(-logits)) + gate_bias[None, :]        # [T,E]
    gsz = E // N_GROUP
    group_scores = scores.reshape(T, N_GROUP, gsz).max(-1)             # [T,G]
    sel_groups = np.argsort(-group_scores, axis=1, kind="stable")[:, :TOPK_GROUP]
    gmask = np.zeros((T, N_GROUP), np.float32)
    gmask[np.arange(T)[:, None], sel_groups] = 1.0
    mask = np.repeat(gmask, gsz, axis=1)                               # [T,E]
    masked = scores * mask
    idx = np.argsort(-masked, axis=1, kind="stable")[:, :TOP_K]        # [T,k]
    w = np.take_along_axis(masked, idx, axis=1)
    w = w / (w.sum(-1, keepdims=True) + 1e-6)
    return idx, w


def _round_up(v, m):
    return ((v + m - 1) // m) * m


def _chunking(C):
    """Return (nch, W) with nch*W == C_padded, W <= 512, W % 64 == 0."""
    nch = max(1, (C + 511) // 512)
    W = _round_up((C + nch - 1) // nch, 64)
    return nch, W


# ----------------------------------------------------------------------------
# Packing helpers: all produce flat bf16 arrays in device consumption order
# ----------------------------------------------------------------------------
def _pack_gu(M, col0, w):
    """M: [I', H] weight (gate or up). Slab for i-cols [col0, col0+w):
    layout [128 p(H-row within h-block), 16 h, w] flattened."""
    A = M[col0:col0 + w, :]                       # [w, H]
    A = A.reshape(w, H_TILES, 128)                # [w, h, p]
    A = np.ascontiguousarray(A.transpose(2, 1, 0))  # [p, h, w]
    return A.astype(BF16).reshape(-1)


def _pack_wd(M, n_i, hg_cols):
    """M: [H, I'] down weight. Pack as [n_hg][128 p(I-row within k-block),
    n_i k, hg_cols] flattened; hg_cols = hg_size*128 H-cols per group."""
    n_hg = H // hg_cols
    Bm = M.reshape(n_hg, hg_cols, n_i, 128)       # [hg, j, k, p]
    Bm = np.ascontiguousarray(Bm.transpose(0, 3, 2, 1))  # [hg, p, k, j]
    return Bm.astype(BF16).reshape(-1)


def _pack_xt(Xpad, nch, W):
    """Xpad: [nch*W, H] f32 tokens (zero padded). Returns [nch*128, 16*W] bf16."""
    out = np.empty((nch * 128, H_TILES * W), dtype=BF16)
    for ci in range(nch):
        Xc = Xpad[ci * W:(ci + 1) * W, :]         # [W, H]
        A = Xc.T.reshape(H_TILES, 128, W)         # [h, p, w]
        out[ci * 128:(ci + 1) * 128, :] = (
            np.ascontiguousarray(A.transpose(1, 0, 2)).reshape(128, -1).astype(BF16)
        )
    return out


def _unpack_y(y_arr, nch, W, C_used):
    """y_arr: [nch*128, 16*W] bf16 -> [C_used, H] f32."""
    Y = np.empty((nch * W, H), dtype=np.float32)
    for ci in range(nch):
        A = y_arr[ci * 128:(ci + 1) * 128, :].astype(np.float32).reshape(128, H_TILES, W)
        Yc = A.transpose(1, 0, 2).reshape(H, W)   # [H, W]
        Y[ci * W:(ci + 1) * W, :] = Yc.T
    return Y[:C_used]


# ----------------------------------------------------------------------------
# Layout plan for the flat packed weight tensor
# ----------------------------------------------------------------------------
I_GROUPS = [(0, 3), (3, 3), (6, 3), (9, 2)]      # routed i-tile groups
SH_GROUPS = [(0, 3)]                             # shared i-tile groups
HG_SIZE = 2                                      # stage-B h-tiles per psum group
HG_COLS = HG_SIZE * 128                          # 256
N_HG = H_TILES // HG_SIZE                        # 8


def _build_layout(W0, nch0, W1, nch1):
    """Compute offsets (in elements) into the flat per-core wpack tensor."""
    off = {}
    cur = 0

    def alloc(key, n):
        nonlocal cur
        off[key] = cur
        cur += n

    for s in (0, 1):
        for (igs, igz) in I_GROUPS:
            w = igz * 128
            alloc(f"g{s}_{igs}", 128 * H_TILES * w)
            alloc(f"u{s}_{igs}", 128 * H_TILES * w)
        alloc(f"d{s}", 128 * I_TILES * HG_COLS * N_HG)
    for (igs, igz) in SH_GROUPS:
        w = igz * 128
        alloc(f"sg_{igs}", 128 * H_TILES * w)
        alloc(f"su_{igs}", 128 * H_TILES * w)
    alloc("sd", 128 * SH_I_TILES * HG_COLS * N_HG)
    return off, cur


# ----------------------------------------------------------------------------
# Device kernel builder
# ----------------------------------------------------------------------------
def _emit_job(nc, pools, wp, off, job):
    """One MLP job: out[c] = silu(X@Wg^T)*(X@Wu^T) @ Wd^T  for tokens X.

    job: dict with n_i, i_groups, W, nch, xt (dram), y (dram), key prefix.
    Weights consumed from wp (flat bf16) at offsets in `off`.
    """
    dt = mybir.dt
    n_i = job["n_i"]
    W = job["W"]
    nch = job["nch"]
    xt_d = job["xt"]
    y_d = job["y"]
    kg, ku, kd = job["kg"], job["ku"], job["kd"]

    wpool, xpool, htpool, ypool, spool, psA, psB = pools

    def slab_ap(ofs, length):
        return wp[ofs:ofs + 128 * length].rearrange("(p l) -> p l", p=128)

    # ---- stage A: HT = silu(G)*U, per chunk, bf16 in SBUF
    xts = []
    for ci in range(nch):
        xt = xpool.tile([128, H_TILES * W], dt.bfloat16, tag="xt")
        nc.sync.dma_start(xt[:], xt_d[ci * 128:(ci + 1) * 128, :])
        xts.append(xt)
    hts = []
    for ci in range(nch):
        ht = htpool.tile([128, n_i * W], dt.bfloat16, tag="ht")
        hts.append(ht)

    for (igs, igz) in job["i_groups"]:
        w_ig = igz * 128
        gslab = wpool.tile([128, H_TILES * w_ig], dt.bfloat16, tag="wg")
        nc.sync.dma_start(gslab[:], slab_ap(off[kg + f"_{igs}"], H_TILES * w_ig))
        uslab = wpool.tile([128, H_TILES * w_ig], dt.bfloat16, tag="wu")
        nc.sync.dma_start(uslab[:], slab_ap(off[ku + f"_{igs}"], H_TILES * w_ig))
        for ci in range(nch):
            for t in range(igz):
                gt = psA.tile([128, W], dt.float32, tag="gt")
                ut = psA.tile([128, W], dt.float32, tag="ut")
                for h in range(H_TILES):
                    nc.tensor.matmul(
                        gt[:], gslab[:, h * w_ig + t * 128: h * w_ig + (t + 1) * 128],
                        xts[ci][:, h * W:(h + 1) * W],
                        start=(h == 0), stop=(h == H_TILES - 1))
                for h in range(H_TILES):
                    nc.tensor.matmul(
                        ut[:], uslab[:, h * w_ig + t * 128: h * w_ig + (t + 1) * 128],
                        xts[ci][:, h * W:(h + 1) * W],
                        start=(h == 0), stop=(h == H_TILES - 1))
                sl = spool.tile([128, W], dt.float32, tag="silu")
                nc.scalar.activation(sl[:], gt[:], mybir.ActivationFunctionType.Silu)
                nc.vector.tensor_mul(
                    hts[ci][:, (igs + t) * W:(igs + t + 1) * W], sl[:], ut[:])

    # ---- stage B: Y = HT^T-style down-projection, psum grouped by h-tiles
    yts = []
    for ci in range(nch):
        ysb = ypool.tile([128, H_TILES * W], dt.bfloat16, tag="ysb")
        yts.append(ysb)
    slab_len = n_i * HG_COLS
    for hg in range(N_HG):
        dslab = wpool.tile([128, slab_len], dt.bfloat16, tag="wd")
        nc.sync.dma_start(dslab[:], slab_ap(off[kd] + hg * 128 * slab_len, slab_len))
        for ci in range(nch):
            ps = [psB.tile([128, W], dt.float32, tag="yt", name=f"yt{hg}_{ci}_{i}")
                  for i in range(HG_SIZE)]
            for k in range(n_i):
                for hl in range(HG_SIZE):
                    nc.tensor.matmul(
                        ps[hl][:],
                        dslab[:, k * HG_COLS + hl * 128: k * HG_COLS + (hl + 1) * 128],
                        hts[ci][:, k * W:(k + 1) * W],
                        start=(k == 0), stop=(k == n_i - 1))
            for hl in range(HG_SIZE):
                ht_idx = hg * HG_SIZE + hl
                nc.vector.tensor_copy(
                    yts[ci][:, ht_idx * W:(ht_idx + 1) * W], ps[hl][:])
    for ci in range(nch):
        nc.sync.dma_start(y_d[ci * 128:(ci + 1) * 128, :], yts[ci][:])


def _build_nc(W0, nch0, W1, nch1, off, wpack_elems):
    dt = mybir.dt
    nc = bass.Bass()
    wp = nc.declare_dram_parameter("wpack", [wpack_elems], dt.bfloat16, isOutput=False)
    xt0 = nc.declare_dram_parameter("xt0", [nch0 * 128, H_TILES * W0], dt.bfloat16, isOutput=False)
    xt1 = nc.declare_dram_parameter("xt1", [nch1 * 128, H_TILES * W1], dt.bfloat16, isOutput=False)
    xtf = nc.declare_dram_parameter("xtf", [2 * 128, H_TILES * 512], dt.bfloat16, isOutput=False)
    y0 = nc.declare_dram_parameter("y0", [nch0 * 128, H_TILES * W0], dt.bfloat16, isOutput=True)
    y1 = nc.declare_dram_parameter("y1", [nch1 * 128, H_TILES * W1], dt.bfloat16, isOutput=True)
    ysh = nc.declare_dram_parameter("ysh", [2 * 128, H_TILES * 512], dt.bfloat16, isOutput=True)

    jobs = [
        dict(n_i=I_TILES, i_groups=I_GROUPS, W=W0, nch=nch0, xt=xt0, y=y0,
             kg="g0", ku="u0", kd="d0"),
        dict(n_i=I_TILES, i_groups=I_GROUPS, W=W1, nch=nch1, xt=xt1, y=y1,
             kg="g1", ku="u1", kd="d1"),
        dict(n_i=SH_I_TILES, i_groups=SH_GROUPS, W=512, nch=2, xt=xtf, y=ysh,
             kg="sg", ku="su", kd="sd"),
    ]

    with ExitStack() as ctx:
        tc = ctx.enter_context(tile.TileContext(nc))
        wpool = ctx.enter_context(tc.tile_pool(name="wpool", bufs=2))
        xpool = ctx.enter_context(tc.tile_pool(name="xpool", bufs=2))
        htpool = ctx.enter_context(tc.tile_pool(name="htpool", bufs=2))
        ypool = ctx.enter_context(tc.tile_pool(name="ypool", bufs=2))
        spool = ctx.enter_context(tc.tile_pool(name="spool", bufs=3))
        psA = ctx.enter_context(tc.tile_pool(name="psA", bufs=3, space="PSUM"))
        psB = ctx.enter_context(tc.tile_pool(name="psB", bufs=2, space="PSUM"))
        pools = (wpool, xpool, htpool, ypool, spool, psA, psB)
        for job in jobs:
            _emit_job(nc, pools, wp, off, job)

    _split_excess_waits(nc)
    return nc


# ----------------------------------------------------------------------------
# Main entry
# ----------------------------------------------------------------------------
def kernel(hidden_states, gate_w, gate_bias, w_gate, w_up, w_down,
           sw_gate, sw_up, sw_down):
    global LAST_EXEC_TIME_NS
    x = np.asarray(hidden_states, dtype=np.float32).reshape(T, H)
    gate_w = np.asarray(gate_w, dtype=np.float32)
    gate_bias = np.asarray(gate_bias, dtype=np.float32)

    # 1. routing on host
    idx, wts = _gate_host(x, gate_w, gate_bias)          # [T,4] each
    tok_lists = [np.nonzero((idx == e).any(axis=1))[0] for e in range(E)]
    tok_w = []
    for e in range(E):
        tl = tok_lists[e]
        sel = idx[tl] == e                                # [n,4] one-hot-ish
        tok_w.append((wts[tl] * sel).sum(axis=1) * SCALE)
    counts = np.array([len(t) for t in tok_lists])

    # 2. pair experts big+small across 8 cores
    order = np.argsort(-counts, kind="stable")
    pairs = [(int(order[i]), int(order[E - 1 - i])) for i in range(N_CORES)]
    C0_raw = max(counts[p[0]] for p in pairs)
    C1_raw = max(counts[p[1]] for p in pairs)
    nch0, W0 = _chunking(int(C0_raw))
    nch1, W1 = _chunking(int(C1_raw))
    C0, C1 = nch0 * W0, nch1 * W1

    # 3. pack per-core inputs
    off, wpack_elems = _build_layout(W0, nch0, W1, nch1)
    xtf_all = _pack_xt(x, 2, 512)                         # shared: all tokens
    in_maps = []
    for c in range(N_CORES):
        wpk = np.empty((wpack_elems,), dtype=BF16)
        for s, (Cs, nchs, Ws) in zip((0, 1), ((C0, nch0, W0), (C1, nch1, W1))):
            e = pairs[c][s]
            for (igs, igz) in I_GROUPS:
                w = igz * 128
                wpk[off[f"g{s}_{igs}"]:off[f"g{s}_{igs}"] + 128 * H_TILES * w] = \
                    _pack_gu(w_gate[e], igs * 128, w)
                wpk[off[f"u{s}_{igs}"]:off[f"u{s}_{igs}"] + 128 * H_TILES * w] = \
                    _pack_gu(w_up[e], igs * 128, w)
            dlen = 128 * I_TILES * HG_COLS * N_HG
            wpk[off[f"d{s}"]:off[f"d{s}"] + dlen] = _pack_wd(w_down[e], I_TILES, HG_COLS)
        # shared slices (pad 352 -> 384)
        sgp = np.zeros((SH_PAD, H), np.float32)
        sup = np.zeros((SH_PAD, H), np.float32)
        sdp = np.zeros((H, SH_PAD), np.float32)
        sl = slice(c * SH_SLICE, (c + 1) * SH_SLICE)
        sgp[:SH_SLICE] = sw_gate[sl]
        sup[:SH_SLICE] = sw_up[sl]
        sdp[:, :SH_SLICE] = sw_down[:, sl]
        for (igs, igz) in SH_GROUPS:
            w = igz * 128
            wpk[off[f"sg_{igs}"]:off[f"sg_{igs}"] + 128 * H_TILES * w] = _pack_gu(sgp, igs * 128, w)
            wpk[off[f"su_{igs}"]:off[f"su_{igs}"] + 128 * H_TILES * w] = _pack_gu(sup, igs * 128, w)
        sdlen = 128 * SH_I_TILES * HG_COLS * N_HG
        wpk[off["sd"]:off["sd"] + sdlen] = _pack_wd(sdp, SH_I_TILES, HG_COLS)

        im = {"wpack": wpk, "xtf": xtf_all}
        for s, (Cs, nchs, Ws) in zip((0, 1), ((C0, nch0, W0), (C1, nch1, W1))):
            e = pairs[c][s]
            Xp = np.zeros((Cs, H), np.float32)
            Xp[:counts[e]] = x[tok_lists[e]]
            im[f"xt{s}"] = _pack_xt(Xp, nchs, Ws)
        in_maps.append(im)

    # 4. build + run
    trace = bool(os.environ.get("BASS_KERNEL_TRACE"))
    if trace:
        _install_axon_profile_shim()
    nc = _build_nc(W0, nch0, W1, nch1, off, wpack_elems)
    res = run_bass_kernel_spmd(nc, in_maps, core_ids=list(range(N_CORES)),
                               trace=trace)
    LAST_EXEC_TIME_NS = res.exec_time_ns

    # 5. unshard: scatter-add routed, sum shared partials
    out = np.zeros((T, H), np.float32)
    for c in range(N_CORES):
        for s, (nchs, Ws) in zip((0, 1), ((nch0, W0), (nch1, W1))):
            e = pairs[c][s]
            n = counts[e]
            if n == 0:
                continue
            Y = _unpack_y(res.results[c][f"y{s}"], nchs, Ws, n)   # [n, H]
            out[tok_lists[e]] += tok_w[e][:, None] * Y
        out += _unpack_y(res.results[c]["ysh"], 2, 512, T)
    return out.reshape(B, S, H).astype(np.float32)
